# revision 17
# baseline (speedup 1.0000x reference)
"""Trainium2 Bass kernel for nn_DagnabbitAutoEncoder.

DAG MLP encoder: 65536 trunk nodes, each gathers 2 parent embeddings (D=128)
from a growing buffer, applies a per-type MLP [256 -> 256 gelu -> 128], and
appends its embedding. Levelized: nodes at the same DAG depth are independent,
so each level is a batched gather + batched per-type matmul.

Strategy (single-core, replicated across the 8 NeuronCores):
- Host: levelize the DAG; within each level sort nodes (non-leaf first, then
  type). Only non-leaf nodes (ever referenced as a parent; ~2/3) get "token"
  slots in an SBUF-resident bf16 embedding table laid out for gpsimd
  dma_gather (token i -> partition i%128, rank i//128, 256B payload).
- Device, per level: dma_gather(transpose=True) pulls parent embeddings
  directly in feature-major layout (x^T) from the SBUF table; per-type matmuls
  on the PE (bf16, fp32 PSUM accumulation); gelu+b1 on the scalar engine
  (per-partition bias, fp32->bf16 out); second matmul; +b2 on the vector
  engine; fp32 y^T DMAed to HBM (transposed output, host untransposes);
  non-leaf bf16 y^T written back into the table via one X-bar DMA transpose
  per level.
- int16 gather indices address all tokens by encoding tok-32768 against a
  table view based at token 32768 (signed offsets address backward). The
  gather skips *trailing* negative indices, so every chunk ends with >=16 pad
  positions encoding token 32768 (a guaranteed-nonnegative tail).
"""

import numpy as np
import ml_dtypes

N_TRUNK = 65536
D = 128
NUM_ROOT = 64
NUM_TYPES = 4
HID = 256

TOK_BASE = 128     # tokens 0..63 = roots, 64..127 unused (trunk 128-aligned)
MID = 32768        # gather view midpoint token (int16 idx = tok - MID)
CREAL = 880        # real nodes per gather chunk (<=896 idx per dma_gather call)
SUB = 512          # matmul/PSUM subchunk columns

bf16 = ml_dtypes.bfloat16

# Populated by kernel() with the BassKernelResults of the last run.
LAST_RESULTS = None


def _rup(x, m):
    return -(-x // m) * m


def _build_schedule(idx, types):
    """Levelize + order nodes; assign table tokens (non-leaf only) and output
    columns (all nodes, chunked with gather pad holes)."""
    lvl = np.zeros(NUM_ROOT + N_TRUNK, dtype=np.int32)
    il = idx.tolist()
    for i in range(N_TRUNK):
        a, b = il[i]
        la = lvl[a]
        lb = lvl[b]
        lvl[NUM_ROOT + i] = (la if la >= lb else lb) + 1
    node_lv = lvl[NUM_ROOT:]
    nlev = int(node_lv.max()) + 1

    referenced = np.zeros(NUM_ROOT + N_TRUNK, dtype=bool)
    referenced[idx.ravel()] = True
    is_leaf = (~referenced[NUM_ROOT:]).astype(np.int8)

    order = np.lexsort((types, is_leaf, node_lv))
    counts = np.bincount(node_lv, minlength=nlev)

    tok = np.full(N_TRUNK, -1, dtype=np.int64)       # table token (non-leaf)
    col = np.empty(N_TRUNK, dtype=np.int64)          # output column (all)
    levels = []
    tok_pos = TOK_BASE
    col_pos = 0
    c0 = 0
    for L in range(1, nlev):
        n = int(counts[L])
        nodes = order[c0:c0 + n]
        c0 += n
        nl = int((is_leaf[nodes] == 0).sum())        # non-leaf count (first nl)
        tok[nodes[:nl]] = tok_pos + np.arange(nl)

        # chunks of <= CREAL real nodes; gather len = rup(real+16, 128)
        chunks = []
        a = 0
        while a < n:
            b = min(a + CREAL, n)
            glen = _rup(b - a + 16, 128)
            col[nodes[a:b]] = col_pos + np.arange(b - a)
            chunks.append((a, b, col_pos, glen))
            col_pos += glen
            a = b

        # subchunk records: (chunk_idx, la, lb, type, leaf, tokoff)
        # la/lb relative to the chunk start; tokoff = token-space offset of
        # the subchunk's first node within the level's ybf tile.
        t_of = types[nodes]
        lf_of = is_leaf[nodes]
        subs = []
        for ci, (a, b, cb, glen) in enumerate(chunks):
            s = a
            while s < b:
                t = int(t_of[s])
                lf = int(lf_of[s])
                e = s
                while e < b and t_of[e] == t and lf_of[e] == lf:
                    e += 1
                subs.append((ci, s - a, e - a, t, lf, s))
                s = e
        levels.append(dict(tok_start=tok_pos, nl=nl, n=n, nodes=nodes,
                           chunks=chunks, subs=subs))
        tok_pos += _rup(nl, 128) if nl else 0
    tokpad = tok_pos
    colpad = col_pos
    assert tokpad <= 65536, tokpad
    par_tok = np.where(idx < NUM_ROOT, idx,
                       tok[np.clip(idx - NUM_ROOT, 0, N_TRUNK - 1)])
    assert par_tok.min() >= 0
    return levels, col, par_tok.astype(np.int64), tokpad, colpad


def _wrap_idx(enc):
    n = len(enc)
    a = np.asarray(enc, dtype=np.int16).reshape(n // 16, 16).T
    return np.tile(a, (8, 1))


def _build_program(levels, par_tok, tokpad, colpad):
    import os
    import concourse.bacc as bacc
    import concourse.tile as tile
    from concourse import mybir
    from concourse.tile import add_dep_helper

    max_levels = int(os.environ.get("KERNEL_MAX_LEVELS", "0"))
    if max_levels > 0:
        levels = levels[:max_levels]
    dbg_skip = set(os.environ.get("KERNEL_SKIP", "").split(","))

    # table must extend past the midpoint so pad token MID exists
    ranks_total = max(tokpad, MID + 128) // 128

    # ---- host-side gather index stream ------------------------------------
    idx_blocks = []
    gather_plan = []  # per level: list of (slot, chunk_idx, glen, idx_col_off)
    col_off = 0
    for lv in levels:
        plan = []
        nodes = lv["nodes"]
        for slot in (0, 1):
            for ci, (a, b, cb, glen) in enumerate(lv["chunks"]):
                enc = np.zeros(glen, dtype=np.int16)  # pad = token MID -> 0
                enc[:b - a] = (par_tok[nodes[a:b], slot] - MID).astype(np.int16)
                idx_blocks.append(_wrap_idx(enc))
                plan.append((slot, ci, glen, col_off))
                col_off += glen // 16
        gather_plan.append(plan)
    idx_img = np.concatenate(idx_blocks, axis=1)
    IDXCOLS = idx_img.shape[1]

    nc = bacc.Bacc("TRN2", target_bir_lowering=False, debug=False)
    dt = mybir.dt

    tab_d = nc.dram_tensor("tab_init", [128, ranks_total * 128], dt.bfloat16, kind="ExternalInput").ap()
    idx_d = nc.dram_tensor("idx_all", [128, IDXCOLS], dt.int16, kind="ExternalInput").ap()
    w1_d = nc.dram_tensor("w1_img", [128, NUM_TYPES * 2 * HID], dt.bfloat16, kind="ExternalInput").ap()
    w2_d = nc.dram_tensor("w2_img", [128, NUM_TYPES * 2 * D], dt.bfloat16, kind="ExternalInput").ap()
    b1_d = nc.dram_tensor("b1_img", [128, NUM_TYPES * 2], dt.float32, kind="ExternalInput").ap()
    b2_d = nc.dram_tensor("b2_img", [128, NUM_TYPES], dt.float32, kind="ExternalInput").ap()
    out_d = nc.dram_tensor("out_t", [128, colpad], dt.float32, kind="ExternalOutput").ap()

    GMAX = max(g for lv in levels for (_, _, _, g) in lv["chunks"])
    WMAX = max(_rup(lv["nl"], 128) for lv in levels)

    with tile.TileContext(nc) as tc:
        with (
            tc.tile_pool(name="const", bufs=1) as cpool,
            tc.tile_pool(name="xs", bufs=3) as xpool,
            tc.tile_pool(name="hs", bufs=3) as hpool,
            tc.tile_pool(name="ys", bufs=4) as ypool,
            tc.tile_pool(name="ybfp", bufs=1) as ybfpool,
            tc.tile_pool(name="ps", bufs=2, space="PSUM") as ppool,
        ):
            tab = cpool.tile([128, ranks_total * 128], dt.bfloat16, tag="tab")
            nc.sync.dma_start(tab[:], tab_d)
            idxt = cpool.tile([128, IDXCOLS], dt.int16, tag="idx")
            nc.sync.dma_start(idxt[:], idx_d)
            w1t = cpool.tile([128, NUM_TYPES * 2 * HID], dt.bfloat16, tag="w1")
            nc.sync.dma_start(w1t[:], w1_d)
            w2t = cpool.tile([128, NUM_TYPES * 2 * D], dt.bfloat16, tag="w2")
            nc.sync.dma_start(w2t[:], w2_d)
            b1t = cpool.tile([128, NUM_TYPES * 2], dt.float32, tag="b1")
            nc.sync.dma_start(b1t[:], b1_d)
            b2t = cpool.tile([128, NUM_TYPES], dt.float32, tag="b2")
            nc.sync.dma_start(b2t[:], b2_d)

            mid_view = tab[:, MID:]
            prev_xbar = None

            for li, lv in enumerate(levels):
                xtiles = {}
                for gi, (slot, ci, glen, icol) in enumerate(gather_plan[li]):
                    xt = xpool.tile([128, GMAX], dt.bfloat16, tag=f"x{slot}", name=f"x{slot}")
                    g = nc.gpsimd.dma_gather(
                        out_ap=xt[:, :glen].rearrange("p (c n) -> p c n", c=1),
                        in_ap=mid_view,
                        idxs_ap=idxt[:, icol:icol + glen // 16],
                        num_idxs=glen,
                        num_idxs_reg=glen,
                        elem_size=D,
                        transpose=True,
                        sbuf_tokens_per_rank=128,
                        sbuf_free_dim_per_rank=256,
                        sbuf_free_dim_pad_per_rank=0,
                        sbuf_byte_offset=0,
                    )
                    if prev_xbar is not None:
                        # the gather reads the whole table via signed offsets;
                        # its declared AP only covers [MID:], so order it
                        # after the previous table write explicitly.
                        add_dep_helper(g.ins, prev_xbar.ins, reason="gather after table write")
                    xtiles[(slot, ci)] = xt

                wl = _rup(lv["nl"], 128)
                ybf = ybfpool.tile([128, WMAX], dt.bfloat16, tag="ybf", name="ybf") if wl else None

                for (ci, la, lb, t, leaf, tokoff) in (() if "compute" in dbg_skip else lv["subs"]):
                    a = la
                    while a < lb:
                        b = min(a + SUB, lb)
                        ncols = b - a
                        xs = (xtiles[(0, ci)], xtiles[(1, ci)])

                        hp = [ppool.tile([128, SUB], dt.float32, tag=f"hp{m}", name=f"hp{m}") for m in (0, 1)]
                        for mh in (0, 1):
                            for k in (0, 1):
                                nc.tensor.matmul(
                                    hp[mh][:, :ncols],
                                    lhsT=w1t[:, (t * 2 + k) * HID + mh * 128:(t * 2 + k) * HID + mh * 128 + 128],
                                    rhs=xs[k][:, a:b],
                                    start=(k == 0),
                                    stop=(k == 1),
                                )
                        hsb = [hpool.tile([128, SUB], dt.bfloat16, tag=f"h{m}", name=f"h{m}") for m in (0, 1)]
                        for mh in (0, 1):
                            nc.scalar.activation(
                                hsb[mh][:, :ncols],
                                hp[mh][:, :ncols],
                                mybir.ActivationFunctionType.Gelu,
                                bias=b1t[:, t * 2 + mh:t * 2 + mh + 1],
                            )
                        yp = ppool.tile([128, SUB], dt.float32, tag="yp")
                        for k in (0, 1):
                            nc.tensor.matmul(
                                yp[:, :ncols],
                                lhsT=w2t[:, (t * 2 + k) * D:(t * 2 + k) * D + D],
                                rhs=hsb[k][:, :ncols],
                                start=(k == 0),
                                stop=(k == 1),
                            )
                        ysb = ypool.tile([128, SUB], dt.float32, tag="ysb")
                        nc.vector.tensor_scalar_add(ysb[:, :ncols], yp[:, :ncols], b2t[:, t:t + 1])
                        cb = lv["chunks"][ci][2]
                        nc.sync.dma_start(out_d[:, cb + a:cb + b], ysb[:, :ncols])
                        if not leaf:
                            to = tokoff + (a - la)
                            nc.vector.tensor_copy(ybf[:, to:to + ncols], ysb[:, :ncols])
                        a = b

                if wl and "xbar" not in dbg_skip and "compute" not in dbg_skip:
                    r0 = lv["tok_start"] // 128
                    prev_xbar = nc.sync.dma_start_transpose(
                        out=tab[:, r0 * 128:(r0 + wl // 128) * 128].rearrange("p (s j) -> p s j", j=128),
                        in_=ybf[:, :wl],
                    )

    nc.compile()
    return nc, idx_img, ranks_total


def kernel(trunk_node_inputs_indices, trunk_node_types, root_node_embeddings,
           W1, b1, W2, b2):
    import os
    from concourse.bass_utils import run_bass_kernel_spmd

    idx = np.asarray(trunk_node_inputs_indices, dtype=np.int32)
    types = np.asarray(trunk_node_types, dtype=np.int32)
    roots = np.asarray(root_node_embeddings, dtype=np.float32)
    W1 = np.asarray(W1, dtype=np.float32)
    b1 = np.asarray(b1, dtype=np.float32)
    W2 = np.asarray(W2, dtype=np.float32)
    b2 = np.asarray(b2, dtype=np.float32)

    levels, col, par_tok, tokpad, colpad = _build_schedule(idx, types)
    nc, idx_img, ranks_total = _build_program(levels, par_tok, tokpad, colpad)

    tab_img = np.zeros((128, ranks_total * 128), dtype=bf16)
    tab_img[:NUM_ROOT, :D] = roots.astype(bf16)

    w1_img = np.empty((128, NUM_TYPES * 2 * HID), dtype=bf16)
    w2_img = np.empty((128, NUM_TYPES * 2 * D), dtype=bf16)
    for t in range(NUM_TYPES):
        for k in range(2):
            w1_img[:, (t * 2 + k) * HID:(t * 2 + k + 1) * HID] = W1[t][k * 128:(k + 1) * 128, :].astype(bf16)
            w2_img[:, (t * 2 + k) * D:(t * 2 + k + 1) * D] = W2[t][k * 128:(k + 1) * 128, :].astype(bf16)
    b1_img = np.empty((128, NUM_TYPES * 2), dtype=np.float32)
    for t in range(NUM_TYPES):
        for mh in range(2):
            b1_img[:, t * 2 + mh] = b1[t][mh * 128:(mh + 1) * 128]
    b2_img = np.ascontiguousarray(b2.T)

    in_map = {
        "tab_init": tab_img,
        "idx_all": idx_img,
        "w1_img": w1_img,
        "w2_img": w2_img,
        "b1_img": b1_img,
        "b2_img": b2_img,
    }
    ncores = int(os.environ.get("KERNEL_CORES", "8"))
    res = run_bass_kernel_spmd(nc, [in_map] * ncores, core_ids=list(range(ncores)))
    global LAST_RESULTS
    LAST_RESULTS = res
    out_t = res.results[0]["out_t"]

    full = np.empty((NUM_ROOT + N_TRUNK, D), dtype=np.float32)
    full[:NUM_ROOT] = roots
    full[NUM_ROOT:] = out_t[:, col].T
    return full


# revision 18
# speedup vs baseline: 1.3282x; 1.3282x over previous
"""Trainium2 Bass kernel for nn_DagnabbitAutoEncoder.

DAG MLP encoder: 65536 trunk nodes, each gathers 2 parent embeddings (D=128)
from a growing buffer, applies a per-type MLP [256 -> 256 gelu -> 128], and
appends its embedding. Levelized: nodes at the same DAG depth are independent,
so each level is a batched gather + batched per-type matmul.

Strategy (single-core, replicated across the 8 NeuronCores):
- Host: levelize the DAG; within each level sort nodes (non-leaf first, then
  type). Only non-leaf nodes (ever referenced as a parent; ~2/3) get "token"
  slots in an SBUF-resident bf16 embedding table laid out for gpsimd
  dma_gather (token i -> partition i%128, rank i//128, 256B payload).
- Device, per level: dma_gather(transpose=True) pulls parent embeddings
  directly in feature-major layout (x^T) from the SBUF table; per-type matmuls
  on the PE (bf16, fp32 PSUM accumulation); gelu+b1 on the scalar engine
  (per-partition bias, fp32->bf16 out); second matmul; +b2 on the vector
  engine; fp32 y^T DMAed to HBM (transposed output, host untransposes);
  non-leaf bf16 y^T written back into the table via one X-bar DMA transpose
  per level.
- int16 gather indices address all tokens by encoding tok-32768 against a
  table view based at token 32768 (signed offsets address backward). The
  gather skips *trailing* negative indices, so every chunk ends with >=16 pad
  positions encoding token 32768 (a guaranteed-nonnegative tail).
"""

import numpy as np
import ml_dtypes

N_TRUNK = 65536
D = 128
NUM_ROOT = 64
NUM_TYPES = 4
HID = 256

TOK_BASE = 128     # tokens 0..63 = roots, 64..127 unused (trunk 128-aligned)
MID = 32768        # gather view midpoint token (int16 idx = tok - MID)
CREAL = 880        # real nodes per gather chunk (<=896 idx per dma_gather call)
SUB = 512          # matmul/PSUM subchunk columns

bf16 = ml_dtypes.bfloat16

# Populated by kernel() with the BassKernelResults of the last run.
LAST_RESULTS = None


def _rup(x, m):
    return -(-x // m) * m


def _build_schedule(idx, types):
    """Levelize + order nodes; assign table tokens (non-leaf only) and output
    columns (all nodes, chunked with gather pad holes)."""
    lvl = np.zeros(NUM_ROOT + N_TRUNK, dtype=np.int32)
    il = idx.tolist()
    for i in range(N_TRUNK):
        a, b = il[i]
        la = lvl[a]
        lb = lvl[b]
        lvl[NUM_ROOT + i] = (la if la >= lb else lb) + 1
    node_lv = lvl[NUM_ROOT:]
    nlev = int(node_lv.max()) + 1

    referenced = np.zeros(NUM_ROOT + N_TRUNK, dtype=bool)
    referenced[idx.ravel()] = True
    is_leaf = (~referenced[NUM_ROOT:]).astype(np.int8)

    order = np.lexsort((types, is_leaf, node_lv))
    counts = np.bincount(node_lv, minlength=nlev)

    tok = np.full(N_TRUNK, -1, dtype=np.int64)       # table token (non-leaf)
    col = np.empty(N_TRUNK, dtype=np.int64)          # output column (all)
    levels = []
    tok_pos = TOK_BASE
    col_pos = 0
    c0 = 0
    for L in range(1, nlev):
        n = int(counts[L])
        nodes = order[c0:c0 + n]
        c0 += n
        nl = int((is_leaf[nodes] == 0).sum())        # non-leaf count (first nl)
        tok[nodes[:nl]] = tok_pos + np.arange(nl)

        # chunks of <= CREAL real nodes; gather len = rup(real+16, 128)
        chunks = []
        a = 0
        while a < n:
            b = min(a + CREAL, n)
            glen = _rup(b - a + 16, 128)
            col[nodes[a:b]] = col_pos + np.arange(b - a)
            chunks.append((a, b, col_pos, glen))
            col_pos += glen
            a = b

        # subchunk records: (chunk_idx, la, lb, type, leaf, tokoff)
        # la/lb relative to the chunk start; tokoff = token-space offset of
        # the subchunk's first node within the level's ybf tile.
        t_of = types[nodes]
        lf_of = is_leaf[nodes]
        subs = []
        for ci, (a, b, cb, glen) in enumerate(chunks):
            s = a
            while s < b:
                t = int(t_of[s])
                lf = int(lf_of[s])
                e = s
                while e < b and t_of[e] == t and lf_of[e] == lf:
                    e += 1
                subs.append((ci, s - a, e - a, t, lf, s))
                s = e
        levels.append(dict(tok_start=tok_pos, nl=nl, n=n, nodes=nodes,
                           chunks=chunks, subs=subs))
        tok_pos += _rup(nl, 128) if nl else 0
    tokpad = tok_pos
    colpad = col_pos
    assert tokpad <= 65536, tokpad
    par_tok = np.where(idx < NUM_ROOT, idx,
                       tok[np.clip(idx - NUM_ROOT, 0, N_TRUNK - 1)])
    assert par_tok.min() >= 0
    return levels, col, par_tok.astype(np.int64), tokpad, colpad


def _wrap_idx(enc):
    n = len(enc)
    a = np.asarray(enc, dtype=np.int16).reshape(n // 16, 16).T
    return np.tile(a, (8, 1))


def _build_program(levels, par_tok, tokpad, colpad):
    import os
    import concourse.bacc as bacc
    import concourse.tile as tile
    from concourse import mybir
    from concourse.tile import add_dep_helper

    max_levels = int(os.environ.get("KERNEL_MAX_LEVELS", "0"))
    if max_levels > 0:
        levels = levels[:max_levels]
    dbg_skip = set(os.environ.get("KERNEL_SKIP", "").split(","))

    # table must extend past the midpoint so pad token MID exists
    ranks_total = max(tokpad, MID + 128) // 128

    # ---- host-side gather index stream ------------------------------------
    idx_blocks = []
    gather_plan = []  # per level: list of (slot, chunk_idx, glen, idx_col_off)
    col_off = 0
    for lv in levels:
        plan = []
        nodes = lv["nodes"]
        for slot in (0, 1):
            for ci, (a, b, cb, glen) in enumerate(lv["chunks"]):
                enc = np.zeros(glen, dtype=np.int16)  # pad = token MID -> 0
                enc[:b - a] = (par_tok[nodes[a:b], slot] - MID).astype(np.int16)
                idx_blocks.append(_wrap_idx(enc))
                plan.append((slot, ci, glen, col_off))
                col_off += glen // 16
        gather_plan.append(plan)
    idx_img = np.concatenate(idx_blocks, axis=1)
    IDXCOLS = idx_img.shape[1]

    nc = bacc.Bacc("TRN2", target_bir_lowering=False, debug=False, num_swdge_queues=2)
    dt = mybir.dt

    tab_d = nc.dram_tensor("tab_init", [128, ranks_total * 128], dt.bfloat16, kind="ExternalInput").ap()
    idx_d = nc.dram_tensor("idx_all", [128, IDXCOLS], dt.int16, kind="ExternalInput").ap()
    w1_d = nc.dram_tensor("w1_img", [128, NUM_TYPES * 2 * HID], dt.bfloat16, kind="ExternalInput").ap()
    w2_d = nc.dram_tensor("w2_img", [128, NUM_TYPES * 2 * D], dt.bfloat16, kind="ExternalInput").ap()
    b1_d = nc.dram_tensor("b1_img", [128, NUM_TYPES * 2], dt.float32, kind="ExternalInput").ap()
    b2_d = nc.dram_tensor("b2_img", [128, NUM_TYPES], dt.float32, kind="ExternalInput").ap()
    out_d = nc.dram_tensor("out_t", [128, colpad], dt.float32, kind="ExternalOutput").ap()

    GMAX = max(g for lv in levels for (_, _, _, g) in lv["chunks"])
    WMAX = max(_rup(lv["nl"], 128) for lv in levels)

    with tile.TileContext(nc) as tc:
        with (
            tc.tile_pool(name="const", bufs=1) as cpool,
            tc.tile_pool(name="xs", bufs=3) as xpool,
            tc.tile_pool(name="hs", bufs=3) as hpool,
            tc.tile_pool(name="ys", bufs=4) as ypool,
            tc.tile_pool(name="ybfp", bufs=1) as ybfpool,
            tc.tile_pool(name="ps", bufs=2, space="PSUM") as ppool,
        ):
            tab = cpool.tile([128, ranks_total * 128], dt.bfloat16, tag="tab")
            nc.sync.dma_start(tab[:], tab_d)
            idxt = cpool.tile([128, IDXCOLS], dt.int16, tag="idx")
            nc.sync.dma_start(idxt[:], idx_d)
            w1t = cpool.tile([128, NUM_TYPES * 2 * HID], dt.bfloat16, tag="w1")
            nc.sync.dma_start(w1t[:], w1_d)
            w2t = cpool.tile([128, NUM_TYPES * 2 * D], dt.bfloat16, tag="w2")
            nc.sync.dma_start(w2t[:], w2_d)
            b1t = cpool.tile([128, NUM_TYPES * 2], dt.float32, tag="b1")
            nc.sync.dma_start(b1t[:], b1_d)
            b2t = cpool.tile([128, NUM_TYPES], dt.float32, tag="b2")
            nc.sync.dma_start(b2t[:], b2_d)

            mid_view = tab[:, MID:]
            prev_xbar = None

            for li, lv in enumerate(levels):
                xtiles = {}
                for gi, (slot, ci, glen, icol) in enumerate(gather_plan[li]):
                    xt = xpool.tile([128, GMAX], dt.bfloat16, tag=f"x{slot}", name=f"x{slot}")
                    g = nc.gpsimd.dma_gather(
                        out_ap=xt[:, :glen].rearrange("p (c n) -> p c n", c=1),
                        in_ap=mid_view,
                        idxs_ap=idxt[:, icol:icol + glen // 16],
                        num_idxs=glen,
                        num_idxs_reg=glen,
                        elem_size=D,
                        transpose=True,
                        sbuf_tokens_per_rank=128,
                        sbuf_free_dim_per_rank=256,
                        sbuf_free_dim_pad_per_rank=0,
                        sbuf_byte_offset=0,
                        queue_num=slot,
                    )
                    if prev_xbar is not None:
                        # the gather reads the whole table via signed offsets;
                        # its declared AP only covers [MID:], so order it
                        # after the previous table write explicitly.
                        add_dep_helper(g.ins, prev_xbar.ins, reason="gather after table write")
                    xtiles[(slot, ci)] = xt

                wl = _rup(lv["nl"], 128)
                ybf = ybfpool.tile([128, WMAX], dt.bfloat16, tag="ybf", name="ybf") if wl else None

                for (ci, la, lb, t, leaf, tokoff) in (() if "compute" in dbg_skip else lv["subs"]):
                    a = la
                    while a < lb:
                        b = min(a + SUB, lb)
                        ncols = b - a
                        xs = (xtiles[(0, ci)], xtiles[(1, ci)])

                        hp = [ppool.tile([128, SUB], dt.float32, tag=f"hp{m}", name=f"hp{m}") for m in (0, 1)]
                        for mh in (0, 1):
                            for k in (0, 1):
                                nc.tensor.matmul(
                                    hp[mh][:, :ncols],
                                    lhsT=w1t[:, (t * 2 + k) * HID + mh * 128:(t * 2 + k) * HID + mh * 128 + 128],
                                    rhs=xs[k][:, a:b],
                                    start=(k == 0),
                                    stop=(k == 1),
                                )
                        hsb = [hpool.tile([128, SUB], dt.bfloat16, tag=f"h{m}", name=f"h{m}") for m in (0, 1)]
                        for mh in (0, 1):
                            nc.scalar.activation(
                                hsb[mh][:, :ncols],
                                hp[mh][:, :ncols],
                                mybir.ActivationFunctionType.Gelu,
                                bias=b1t[:, t * 2 + mh:t * 2 + mh + 1],
                            )
                        yp = ppool.tile([128, SUB], dt.float32, tag="yp")
                        for k in (0, 1):
                            nc.tensor.matmul(
                                yp[:, :ncols],
                                lhsT=w2t[:, (t * 2 + k) * D:(t * 2 + k) * D + D],
                                rhs=hsb[k][:, :ncols],
                                start=(k == 0),
                                stop=(k == 1),
                            )
                        ysb = ypool.tile([128, SUB], dt.float32, tag="ysb")
                        nc.vector.tensor_scalar_add(ysb[:, :ncols], yp[:, :ncols], b2t[:, t:t + 1])
                        cb = lv["chunks"][ci][2]
                        nc.sync.dma_start(out_d[:, cb + a:cb + b], ysb[:, :ncols])
                        if not leaf:
                            to = tokoff + (a - la)
                            nc.vector.tensor_copy(ybf[:, to:to + ncols], ysb[:, :ncols])
                        a = b

                if wl and "xbar" not in dbg_skip and "compute" not in dbg_skip:
                    r0 = lv["tok_start"] // 128
                    prev_xbar = nc.sync.dma_start_transpose(
                        out=tab[:, r0 * 128:(r0 + wl // 128) * 128].rearrange("p (s j) -> p s j", j=128),
                        in_=ybf[:, :wl],
                    )

    nc.compile()
    return nc, idx_img, ranks_total


def kernel(trunk_node_inputs_indices, trunk_node_types, root_node_embeddings,
           W1, b1, W2, b2):
    import os
    from concourse.bass_utils import run_bass_kernel_spmd

    idx = np.asarray(trunk_node_inputs_indices, dtype=np.int32)
    types = np.asarray(trunk_node_types, dtype=np.int32)
    roots = np.asarray(root_node_embeddings, dtype=np.float32)
    W1 = np.asarray(W1, dtype=np.float32)
    b1 = np.asarray(b1, dtype=np.float32)
    W2 = np.asarray(W2, dtype=np.float32)
    b2 = np.asarray(b2, dtype=np.float32)

    levels, col, par_tok, tokpad, colpad = _build_schedule(idx, types)
    nc, idx_img, ranks_total = _build_program(levels, par_tok, tokpad, colpad)

    tab_img = np.zeros((128, ranks_total * 128), dtype=bf16)
    tab_img[:NUM_ROOT, :D] = roots.astype(bf16)

    w1_img = np.empty((128, NUM_TYPES * 2 * HID), dtype=bf16)
    w2_img = np.empty((128, NUM_TYPES * 2 * D), dtype=bf16)
    for t in range(NUM_TYPES):
        for k in range(2):
            w1_img[:, (t * 2 + k) * HID:(t * 2 + k + 1) * HID] = W1[t][k * 128:(k + 1) * 128, :].astype(bf16)
            w2_img[:, (t * 2 + k) * D:(t * 2 + k + 1) * D] = W2[t][k * 128:(k + 1) * 128, :].astype(bf16)
    b1_img = np.empty((128, NUM_TYPES * 2), dtype=np.float32)
    for t in range(NUM_TYPES):
        for mh in range(2):
            b1_img[:, t * 2 + mh] = b1[t][mh * 128:(mh + 1) * 128]
    b2_img = np.ascontiguousarray(b2.T)

    in_map = {
        "tab_init": tab_img,
        "idx_all": idx_img,
        "w1_img": w1_img,
        "w2_img": w2_img,
        "b1_img": b1_img,
        "b2_img": b2_img,
    }
    ncores = int(os.environ.get("KERNEL_CORES", "8"))
    res = run_bass_kernel_spmd(nc, [in_map] * ncores, core_ids=list(range(ncores)))
    global LAST_RESULTS
    LAST_RESULTS = res
    out_t = res.results[0]["out_t"]

    full = np.empty((NUM_ROOT + N_TRUNK, D), dtype=np.float32)
    full[:NUM_ROOT] = roots
    full[NUM_ROOT:] = out_t[:, col].T
    return full


# revision 19
# speedup vs baseline: 1.5082x; 1.1355x over previous
"""Trainium2 Bass kernel for nn_DagnabbitAutoEncoder.

DAG MLP encoder: 65536 trunk nodes, each gathers 2 parent embeddings (D=128)
from a growing buffer, applies a per-type MLP [256 -> 256 gelu -> 128], and
appends its embedding. Levelized: nodes at the same DAG depth are independent,
so each level is a batched gather + batched per-type matmul.

Strategy (single-core, replicated across the 8 NeuronCores):
- Host: levelize the DAG; within each level sort nodes (non-leaf first, then
  type). Only non-leaf nodes (ever referenced as a parent; ~2/3) get "token"
  slots in an SBUF-resident bf16 embedding table laid out for gpsimd
  dma_gather (token i -> partition i%128, rank i//128, 256B payload).
- Device, per level: dma_gather(transpose=True) pulls parent embeddings
  directly in feature-major layout (x^T) from the SBUF table; per-type matmuls
  on the PE (bf16, fp32 PSUM accumulation); gelu+b1 on the scalar engine
  (per-partition bias, fp32->bf16 out); second matmul; +b2 on the vector
  engine; fp32 y^T DMAed to HBM (transposed output, host untransposes);
  non-leaf bf16 y^T written back into the table via one X-bar DMA transpose
  per level.
- int16 gather indices address all tokens by encoding tok-32768 against a
  table view based at token 32768 (signed offsets address backward). The
  gather skips *trailing* negative indices, so every chunk ends with >=16 pad
  positions encoding token 32768 (a guaranteed-nonnegative tail).
"""

import numpy as np
import ml_dtypes

N_TRUNK = 65536
D = 128
NUM_ROOT = 64
NUM_TYPES = 4
HID = 256

TOK_BASE = 128     # tokens 0..63 = roots, 64..127 unused (trunk 128-aligned)
MID = 32768        # gather view midpoint token (int16 idx = tok - MID)
CREAL = 880        # real nodes per gather chunk (<=896 idx per dma_gather call)
SUB = 512          # matmul/PSUM subchunk columns

bf16 = ml_dtypes.bfloat16

# Populated by kernel() with the BassKernelResults of the last run.
LAST_RESULTS = None


def _rup(x, m):
    return -(-x // m) * m


def _build_schedule(idx, types):
    """Levelize + order nodes; assign table tokens (non-leaf only) and output
    columns (all nodes, chunked with gather pad holes)."""
    lvl = np.zeros(NUM_ROOT + N_TRUNK, dtype=np.int32)
    il = idx.tolist()
    for i in range(N_TRUNK):
        a, b = il[i]
        la = lvl[a]
        lb = lvl[b]
        lvl[NUM_ROOT + i] = (la if la >= lb else lb) + 1
    node_lv = lvl[NUM_ROOT:]
    nlev = int(node_lv.max()) + 1

    referenced = np.zeros(NUM_ROOT + N_TRUNK, dtype=bool)
    referenced[idx.ravel()] = True
    is_leaf = (~referenced[NUM_ROOT:]).astype(np.int8)

    order = np.lexsort((types, is_leaf, node_lv))
    counts = np.bincount(node_lv, minlength=nlev)

    tok = np.full(N_TRUNK, -1, dtype=np.int64)       # table token (non-leaf)
    col = np.empty(N_TRUNK, dtype=np.int64)          # output column (all)
    levels = []
    tok_pos = TOK_BASE
    col_pos = 0
    c0 = 0
    for L in range(1, nlev):
        n = int(counts[L])
        nodes = order[c0:c0 + n]
        c0 += n
        nl = int((is_leaf[nodes] == 0).sum())        # non-leaf count (first nl)
        tok[nodes[:nl]] = tok_pos + np.arange(nl)

        # chunks of <= CREAL real nodes; gather len = rup(real+16, 128)
        chunks = []
        a = 0
        while a < n:
            b = min(a + CREAL, n)
            glen = _rup(b - a + 16, 128)
            col[nodes[a:b]] = col_pos + np.arange(b - a)
            chunks.append((a, b, col_pos, glen))
            col_pos += glen
            a = b

        # subchunk records: (chunk_idx, la, lb, type, leaf, tokoff)
        # la/lb relative to the chunk start; tokoff = token-space offset of
        # the subchunk's first node within the level's ybf tile.
        t_of = types[nodes]
        lf_of = is_leaf[nodes]
        subs = []
        for ci, (a, b, cb, glen) in enumerate(chunks):
            s = a
            while s < b:
                t = int(t_of[s])
                lf = int(lf_of[s])
                e = s
                while e < b and t_of[e] == t and lf_of[e] == lf:
                    e += 1
                subs.append((ci, s - a, e - a, t, lf, s))
                s = e
        levels.append(dict(tok_start=tok_pos, nl=nl, n=n, nodes=nodes,
                           chunks=chunks, subs=subs))
        tok_pos += _rup(nl, 128) if nl else 0
    tokpad = tok_pos
    colpad = col_pos
    assert tokpad <= 65536, tokpad
    par_tok = np.where(idx < NUM_ROOT, idx,
                       tok[np.clip(idx - NUM_ROOT, 0, N_TRUNK - 1)])
    assert par_tok.min() >= 0
    return levels, col, par_tok.astype(np.int64), tokpad, colpad


def _wrap_idx(enc):
    n = len(enc)
    a = np.asarray(enc, dtype=np.int16).reshape(n // 16, 16).T
    return np.tile(a, (8, 1))


def _build_program(levels, par_tok, tokpad, colpad):
    import os
    import concourse.bacc as bacc
    import concourse.tile as tile
    from concourse import mybir
    from concourse.tile import add_dep_helper

    max_levels = int(os.environ.get("KERNEL_MAX_LEVELS", "0"))
    if max_levels > 0:
        levels = levels[:max_levels]
    dbg_skip = set(os.environ.get("KERNEL_SKIP", "").split(","))

    # table must extend past the midpoint so pad token MID exists
    ranks_total = max(tokpad, MID + 128) // 128

    # ---- host-side gather index stream ------------------------------------
    idx_blocks = []
    gather_plan = []  # per level: list of (slot, chunk_idx, glen, idx_col_off)
    col_off = 0
    for lv in levels:
        plan = []
        nodes = lv["nodes"]
        for slot in (0, 1):
            for ci, (a, b, cb, glen) in enumerate(lv["chunks"]):
                enc = np.zeros(glen, dtype=np.int16)  # pad = token MID -> 0
                enc[:b - a] = (par_tok[nodes[a:b], slot] - MID).astype(np.int16)
                idx_blocks.append(_wrap_idx(enc))
                plan.append((slot, ci, glen, col_off))
                col_off += glen // 16
        gather_plan.append(plan)
    idx_img = np.concatenate(idx_blocks, axis=1)
    IDXCOLS = idx_img.shape[1]

    nc = bacc.Bacc("TRN2", target_bir_lowering=False, debug=False, num_swdge_queues=4)
    dt = mybir.dt

    tab_d = nc.dram_tensor("tab_init", [128, ranks_total * 128], dt.bfloat16, kind="ExternalInput").ap()
    idx_d = nc.dram_tensor("idx_all", [128, IDXCOLS], dt.int16, kind="ExternalInput").ap()
    w1_d = nc.dram_tensor("w1_img", [128, NUM_TYPES * 2 * HID], dt.bfloat16, kind="ExternalInput").ap()
    w2_d = nc.dram_tensor("w2_img", [128, NUM_TYPES * 2 * D], dt.bfloat16, kind="ExternalInput").ap()
    b1_d = nc.dram_tensor("b1_img", [128, NUM_TYPES * 2], dt.float32, kind="ExternalInput").ap()
    b2_d = nc.dram_tensor("b2_img", [128, NUM_TYPES], dt.float32, kind="ExternalInput").ap()
    out_d = nc.dram_tensor("out_t", [128, colpad], dt.float32, kind="ExternalOutput").ap()

    GMAX = max(g for lv in levels for (_, _, _, g) in lv["chunks"])
    WMAX = max(_rup(lv["nl"], 128) for lv in levels)

    with tile.TileContext(nc) as tc:
        with (
            tc.tile_pool(name="const", bufs=1) as cpool,
            tc.tile_pool(name="xs", bufs=3) as xpool,
            tc.tile_pool(name="hs", bufs=3) as hpool,
            tc.tile_pool(name="ys", bufs=4) as ypool,
            tc.tile_pool(name="ybfp", bufs=1) as ybfpool,
            tc.tile_pool(name="ps", bufs=2, space="PSUM") as ppool,
        ):
            tab = cpool.tile([128, ranks_total * 128], dt.bfloat16, tag="tab")
            nc.sync.dma_start(tab[:], tab_d)
            idxt = cpool.tile([128, IDXCOLS], dt.int16, tag="idx")
            nc.sync.dma_start(idxt[:], idx_d)
            w1t = cpool.tile([128, NUM_TYPES * 2 * HID], dt.bfloat16, tag="w1")
            nc.sync.dma_start(w1t[:], w1_d)
            w2t = cpool.tile([128, NUM_TYPES * 2 * D], dt.bfloat16, tag="w2")
            nc.sync.dma_start(w2t[:], w2_d)
            b1t = cpool.tile([128, NUM_TYPES * 2], dt.float32, tag="b1")
            nc.sync.dma_start(b1t[:], b1_d)
            b2t = cpool.tile([128, NUM_TYPES], dt.float32, tag="b2")
            nc.sync.dma_start(b2t[:], b2_d)

            mid_view = tab[:, MID:]
            prev_xbar = None

            for li, lv in enumerate(levels):
                xtiles = {}
                for gi, (slot, ci, glen, icol) in enumerate(gather_plan[li]):
                    xt = xpool.tile([128, GMAX], dt.bfloat16, tag=f"x{slot}{ci % 2}", name=f"x{slot}")
                    g = nc.gpsimd.dma_gather(
                        out_ap=xt[:, :glen].rearrange("p (c n) -> p c n", c=1),
                        in_ap=mid_view,
                        idxs_ap=idxt[:, icol:icol + glen // 16],
                        num_idxs=glen,
                        num_idxs_reg=glen,
                        elem_size=D,
                        transpose=True,
                        sbuf_tokens_per_rank=128,
                        sbuf_free_dim_per_rank=256,
                        sbuf_free_dim_pad_per_rank=0,
                        sbuf_byte_offset=0,
                        queue_num=slot * 2 + ci % 2,
                    )
                    if prev_xbar is not None:
                        # the gather reads the whole table via signed offsets;
                        # its declared AP only covers [MID:], so order it
                        # after the previous table write explicitly.
                        add_dep_helper(g.ins, prev_xbar.ins, reason="gather after table write")
                    xtiles[(slot, ci)] = xt

                wl = _rup(lv["nl"], 128)
                ybf = ybfpool.tile([128, WMAX], dt.bfloat16, tag="ybf", name="ybf") if wl else None

                for (ci, la, lb, t, leaf, tokoff) in (() if "compute" in dbg_skip else lv["subs"]):
                    a = la
                    while a < lb:
                        b = min(a + SUB, lb)
                        ncols = b - a
                        xs = (xtiles[(0, ci)], xtiles[(1, ci)])

                        hp = [ppool.tile([128, SUB], dt.float32, tag=f"hp{m}", name=f"hp{m}") for m in (0, 1)]
                        for mh in (0, 1):
                            for k in (0, 1):
                                nc.tensor.matmul(
                                    hp[mh][:, :ncols],
                                    lhsT=w1t[:, (t * 2 + k) * HID + mh * 128:(t * 2 + k) * HID + mh * 128 + 128],
                                    rhs=xs[k][:, a:b],
                                    start=(k == 0),
                                    stop=(k == 1),
                                )
                        hsb = [hpool.tile([128, SUB], dt.bfloat16, tag=f"h{m}", name=f"h{m}") for m in (0, 1)]
                        for mh in (0, 1):
                            nc.scalar.activation(
                                hsb[mh][:, :ncols],
                                hp[mh][:, :ncols],
                                mybir.ActivationFunctionType.Gelu,
                                bias=b1t[:, t * 2 + mh:t * 2 + mh + 1],
                            )
                        yp = ppool.tile([128, SUB], dt.float32, tag="yp")
                        for k in (0, 1):
                            nc.tensor.matmul(
                                yp[:, :ncols],
                                lhsT=w2t[:, (t * 2 + k) * D:(t * 2 + k) * D + D],
                                rhs=hsb[k][:, :ncols],
                                start=(k == 0),
                                stop=(k == 1),
                            )
                        ysb = ypool.tile([128, SUB], dt.float32, tag="ysb")
                        nc.vector.tensor_scalar_add(ysb[:, :ncols], yp[:, :ncols], b2t[:, t:t + 1])
                        cb = lv["chunks"][ci][2]
                        nc.sync.dma_start(out_d[:, cb + a:cb + b], ysb[:, :ncols])
                        if not leaf:
                            to = tokoff + (a - la)
                            nc.vector.tensor_copy(ybf[:, to:to + ncols], ysb[:, :ncols])
                        a = b

                if wl and "xbar" not in dbg_skip and "compute" not in dbg_skip:
                    r0 = lv["tok_start"] // 128
                    prev_xbar = nc.sync.dma_start_transpose(
                        out=tab[:, r0 * 128:(r0 + wl // 128) * 128].rearrange("p (s j) -> p s j", j=128),
                        in_=ybf[:, :wl],
                    )

    nc.compile()
    return nc, idx_img, ranks_total


def kernel(trunk_node_inputs_indices, trunk_node_types, root_node_embeddings,
           W1, b1, W2, b2):
    import os
    from concourse.bass_utils import run_bass_kernel_spmd

    idx = np.asarray(trunk_node_inputs_indices, dtype=np.int32)
    types = np.asarray(trunk_node_types, dtype=np.int32)
    roots = np.asarray(root_node_embeddings, dtype=np.float32)
    W1 = np.asarray(W1, dtype=np.float32)
    b1 = np.asarray(b1, dtype=np.float32)
    W2 = np.asarray(W2, dtype=np.float32)
    b2 = np.asarray(b2, dtype=np.float32)

    levels, col, par_tok, tokpad, colpad = _build_schedule(idx, types)
    nc, idx_img, ranks_total = _build_program(levels, par_tok, tokpad, colpad)

    tab_img = np.zeros((128, ranks_total * 128), dtype=bf16)
    tab_img[:NUM_ROOT, :D] = roots.astype(bf16)

    w1_img = np.empty((128, NUM_TYPES * 2 * HID), dtype=bf16)
    w2_img = np.empty((128, NUM_TYPES * 2 * D), dtype=bf16)
    for t in range(NUM_TYPES):
        for k in range(2):
            w1_img[:, (t * 2 + k) * HID:(t * 2 + k + 1) * HID] = W1[t][k * 128:(k + 1) * 128, :].astype(bf16)
            w2_img[:, (t * 2 + k) * D:(t * 2 + k + 1) * D] = W2[t][k * 128:(k + 1) * 128, :].astype(bf16)
    b1_img = np.empty((128, NUM_TYPES * 2), dtype=np.float32)
    for t in range(NUM_TYPES):
        for mh in range(2):
            b1_img[:, t * 2 + mh] = b1[t][mh * 128:(mh + 1) * 128]
    b2_img = np.ascontiguousarray(b2.T)

    in_map = {
        "tab_init": tab_img,
        "idx_all": idx_img,
        "w1_img": w1_img,
        "w2_img": w2_img,
        "b1_img": b1_img,
        "b2_img": b2_img,
    }
    ncores = int(os.environ.get("KERNEL_CORES", "8"))
    res = run_bass_kernel_spmd(nc, [in_map] * ncores, core_ids=list(range(ncores)))
    global LAST_RESULTS
    LAST_RESULTS = res
    out_t = res.results[0]["out_t"]

    full = np.empty((NUM_ROOT + N_TRUNK, D), dtype=np.float32)
    full[:NUM_ROOT] = roots
    full[NUM_ROOT:] = out_t[:, col].T
    return full
